# revision 35
# baseline (speedup 1.0000x reference)
import os
import sys

import numpy as np

for _p in ("/opt/trn_rl_repo",):
    if _p not in sys.path and os.path.isdir(_p):
        sys.path.insert(0, _p)

import ml_dtypes
import bass_rust

from concourse import bacc, mybir, tile
from concourse import bass_utils
from concourse import dve_ops
from concourse.dve_spec import C0, C1, C2, C3, Spec, Src0, Src1, lower, minn, relu
from concourse.dve_spec import _has_src1 as has_src1
from concourse.dve_spec import _spill_c3_to_src1
from concourse.dve_uop import DveOpSpec


def _register_dve_op(name, spec):
    for op in dve_ops.OPS:
        if op.name == name:
            return op
    row = dve_ops._CUSTOM_DVE_ROW_BASE + len(dve_ops.OPS)
    assert row < 0x20
    shas = {}
    for ver in ("v3", "v4"):
        shas[ver] = DveOpSpec(
            name=name, opcode=row, uops=lower(spec, ver=ver), rd1_en=has_src1(spec)
        ).sha(ver)
    op = dve_ops.DveOp(name, spec, subdim=False, uops_sha=shas)
    dve_ops.OPS.append(op)
    dve_ops.CUSTOM_DVE_SPECS[name] = spec
    dve_ops._SUB_OPCODE_FOR_NAME[name] = row
    return op


def _q(t, s0, s1, imm2):
    f = np.float32
    t = np.minimum(np.maximum(t, f(0.0)), f(s0)).astype(np.float32)
    t = (t + f(s1)).astype(np.float32)
    t = (t - f(s1)).astype(np.float32)
    return (t * f(imm2)).astype(np.float32)


QUANT_OP = _register_dve_op(
    "QUANT_CRS_ANT",
    Spec(
        body=(minn(relu(Src0 * C0), C0) + C1 - C1) * C2,
        reference=lambda in0, in1, s0, s1, imm2: _q(
            (in0.astype(np.float32) * np.float32(s0)).astype(np.float32), s0, s1, imm2
        ),
    ),
)

QUANT_AFF_OP = _register_dve_op(
    "QUANT_AFF_ANT",
    Spec(
        body=_spill_c3_to_src1(minn(relu(Src0 * C0 + C3), C1) + C2 - C2),
        reference=lambda in0, in1, s0, s1, imm2: (
            lambda t: (
                (np.minimum(np.maximum(t, np.float32(0.0)), np.float32(s1)).astype(np.float32)
                 + np.float32(imm2)).astype(np.float32)
                - np.float32(imm2)
            ).astype(np.float32)
        )(
            (
                in0.astype(np.float32).reshape(in0.shape[0], -1)
                * s0.astype(np.float32).reshape(-1, 1)
                + in1.astype(np.float32).reshape(-1, 1)
            ).astype(np.float32)
        ).reshape(in0.shape),
    ),
)

QUANT_SIMPLE_OP = _register_dve_op(
    "QUANT_SIM_ANT",
    Spec(
        body=minn(relu(Src0), C0) + C1 - C1,
        reference=lambda in0, in1, s0, s1, imm2: (
            lambda t: (
                (np.minimum(np.maximum(t, np.float32(0.0)), np.float32(s0)).astype(np.float32)
                 + np.float32(s1)).astype(np.float32)
                - np.float32(s1)
            ).astype(np.float32)
        )(in0.astype(np.float32)),
    ),
)

QUANT_RES_OP = _register_dve_op(
    "QUANT_RES_ANT",
    Spec(
        body=(minn(relu((Src0 + Src1) * C0), C0) + C1 - C1) * C2,
        reference=lambda in0, in1, s0, s1, imm2: _q(
            (
                (
                    in0.astype(np.float32).reshape(in0.shape[0], -1)
                    + in1.astype(np.float32).reshape(in1.shape[0], -1)
                ).astype(np.float32)
                * np.float32(s0)
            ).astype(np.float32),
            s0, s1, imm2,
        ).reshape(in0.shape),
    ),
)

N_CORES = 8
P = 128
NIMG = 8
H = W = 28
HP = 30
HR = 31
HF = 14
NCH = NIMG * 2
NRUN = HF * HP
HR2 = HR + 1
MAGIC = float(2 ** 23)
F32 = mybir.dt.float32
F16 = mybir.dt.float16
BF16 = mybir.dt.bfloat16
F8 = mybir.dt.float8e4
AF = mybir.ActivationFunctionType
OP = mybir.AluOpType
DR = mybir.MatmulPerfMode.DoubleRow

CONV_GROUP = 3

DR_PAIRS = [(0, 3), (1, 4), (2, 5), (6, None), (7, None), (8, None)]


def _quant_int(w: np.ndarray) -> np.ndarray:
    t = np.tanh(w.astype(np.float32))
    mx = np.max(np.abs(t))
    tq = t / (np.float32(2.0) * mx) + np.float32(0.5)
    j = np.round(tq * np.float32(15.0))
    return (np.float32(2.0) * j - np.float32(15.0)).astype(np.float32)


def _weights_lhsT(m: np.ndarray, dtype) -> np.ndarray:
    return np.ascontiguousarray(m.transpose(1, 2, 3, 0).reshape(P, 9, P)).astype(dtype)


def _emit(nc, tc):
    x_d = nc.dram_tensor("x", [NIMG, P, H, W], F32, kind="ExternalInput").ap()
    w1_d = nc.dram_tensor("wq1", [P, 9, P], F16, kind="ExternalInput").ap()
    w2p_d = nc.dram_tensor("wq2p", [P, 6, 2, P], F8, kind="ExternalInput").ap()
    gb_d = nc.dram_tensor("gb", [P, 4], F32, kind="ExternalInput").ap()
    eye_d = nc.dram_tensor("eye", [P, P], F32, kind="ExternalInput").ap()
    sm_d = nc.dram_tensor("smat", [2 * N_CORES, 4], F32, kind="ExternalInput").ap()
    out_d = nc.dram_tensor("out", [NIMG, P, H, W], F16, kind="ExternalOutput").ap()
    wu_d = nc.dram_tensor("wu", [16], F32, kind="ExternalOutput").ap()

    rg = [list(range(N_CORES))]

    with (
        tc.tile_pool(name="persist", bufs=1) as pp,
        tc.tile_pool(name="fin", bufs=4) as fp,
        tc.tile_pool(name="cpsum", bufs=6, space="PSUM") as pcp,
        tc.tile_pool(name="tpsum", bufs=1, space="PSUM") as tps,
        tc.tile_pool(name="dram", bufs=1, space="DRAM") as dp,
    ):
        wu_sb = pp.tile([2, 16], F32, tag="wusb")
        wu_in = dp.tile([2, 16], F32, tag="wuin", name="wuin")
        wu_out = dp.tile([N_CORES * 2, 16], F32, tag="wuout", name="wuout")
        nc.vector.memset(wu_sb[:], 1.0)
        nc.gpsimd.dma_start(out=wu_in.opt(), in_=wu_sb[:])
        nc.gpsimd.collective_compute(
            "AllGather", OP.bypass, replica_groups=rg,
            ins=[wu_in.opt()], outs=[wu_out.opt()],
        )

        xpad = pp.tile([P, NIMG, HP, HP], F32, tag="xpad")
        xh = pp.tile([P, NIMG, HR, HP], F16, tag="xh")
        a1f = pp.tile([P, NIMG, HR2, HP], F8, tag="a1f")
        raw1 = pp.tile([P, NIMG, H, W], F32, tag="raw1")
        raw2 = pp.tile([P, NIMG, H, W], F32, tag="raw2")
        w1s = pp.tile([P, 9, P], F16, tag="w1s")
        w2ps = pp.tile([P, 6, 2, P], F8, tag="w2ps")
        gbs = pp.tile([P, 4], F32, tag="gbs")
        eyes = pp.tile([P, P], F32, tag="eyes")
        sms = pp.tile([2 * N_CORES, 4], F32, tag="sms")

        nc.vector.memset(xpad[:, :, 0, :], 0.0)
        nc.vector.memset(xpad[:, :, HP - 1, :], 0.0)
        nc.vector.memset(xpad[:, :, 1 : HP - 1, 0], 0.0)
        nc.vector.memset(xpad[:, :, 1 : HP - 1, HP - 1], 0.0)
        nc.vector.memset(xh[:, :, HR - 1, :], 0.0)
        nc.gpsimd.memset(a1f[:, :, 0, :], 0.0)
        nc.gpsimd.memset(a1f[:, :, HP - 1 :, :], 0.0)
        nc.gpsimd.memset(a1f[:, :, 1 : HP - 1, 0], 0.0)
        nc.gpsimd.memset(a1f[:, :, 1 : HP - 1, HP - 1], 0.0)

        for n in range(NIMG):
            q = nc.sync if n % 2 == 0 else nc.scalar
            q.dma_start(out=xpad[:, n, 1 : 1 + H, 1 : 1 + W], in_=x_d[n])
            nc.vector.tensor_copy(out=xh[:, n, 0:HP, :], in_=xpad[:, n])
            if n == 1:
                nc.scalar.dma_start(out=w1s[:], in_=w1_d)
        nc.sync.dma_start(out=w2ps[:], in_=w2p_d)
        nc.sync.dma_start(out=gbs[:], in_=gb_d)
        nc.sync.dma_start(out=eyes[:], in_=eye_d)
        nc.sync.dma_start(out=sms[:], in_=sm_d)

        warm = pp.tile([P, 1], F32, tag="warm")
        nc.vector.memset(warm[:], 1.0)
        nc.scalar.activation(out=warm[:], in_=warm[:], func=AF.Sqrt)

        def conv(pieces, wsb, rawbuf, stbuf):
            flats = {}
            for pi, piece in enumerate(pieces):
                for n in range(NIMG):
                    flats[(pi, n)] = piece[:, n].rearrange("p h w -> p (h w)")
            nmm = 9 * len(pieces)
            groups = [[0], [1, 2]] + [
                list(range(gs, min(gs + CONV_GROUP, NCH)))
                for gs in range(3, NCH, CONV_GROUP)
            ]
            for grp in groups:
                pt = {
                    ci: pcp.tile([P, HF, HP], F32, tag="cps", name=f"cps{ci}")
                    for ci in grp
                }
                for t in range(9):
                    dy, dx = divmod(t, 3)
                    for pi in range(len(pieces)):
                        k = t * len(pieces) + pi
                        for ci in grp:
                            n, hh = divmod(ci, 2)
                            off = (hh * HF + dy) * HP + dx
                            nc.tensor.matmul(
                                pt[ci][:],
                                wsb[:, t, :],
                                flats[(pi, n)][:, off : off + NRUN],
                                start=(k == 0),
                                stop=(k == nmm - 1),
                            )
                for ci in grp:
                    n, hh = divmod(ci, 2)
                    h0 = hh * HF
                    if ci == NCH - 1:
                        nc.vector.tensor_copy(
                            out=rawbuf[:, n, h0 : h0 + HF, :],
                            in_=pt[ci][:, :, 0:W],
                        )
                    else:
                        nc.scalar.activation(
                            out=rawbuf[:, n, h0 : h0 + HF, :],
                            in_=pt[ci][:, :, 0:W],
                            func=AF.Copy,
                        )
                    nc.vector.bn_stats(
                        out=stbuf[:, 6 * ci : 6 * (ci + 1)],
                        in_=rawbuf[:, n, h0 : h0 + HF, :].rearrange("p h w -> p (h w)"),
                    )

        def conv2_dr(piece, wp, rawbuf, stbuf):
            flats = {n: piece[:, n].rearrange("p h w -> p (h w)") for n in range(NIMG)}
            groups = [[0], [1, 2]] + [
                list(range(gs, min(gs + CONV_GROUP, NCH)))
                for gs in range(3, NCH, CONV_GROUP)
            ]
            for grp in groups:
                pt = {
                    ci: pcp.tile([P, HF, HP], F32, tag="cps", name=f"cps{ci}")
                    for ci in grp
                }
                for pi, (ta, _tb) in enumerate(DR_PAIRS):
                    dy, dx = divmod(ta, 3)
                    for ci in grp:
                        n, hh = divmod(ci, 2)
                        off = (hh * HF + dy) * HP + dx
                        mv = flats[n][:, off : off + NRUN].copy()
                        raw = mv.ap.to_list()
                        mv.ap = bass_rust.VecI64Pair([raw[0], [HP, 2], [1, NRUN]])
                        nc.tensor.matmul(
                            pt[ci][:], wp[:, pi], mv,
                            start=(pi == 0), stop=(pi == len(DR_PAIRS) - 1),
                            perf_mode=DR,
                        )
                for ci in grp:
                    n, hh = divmod(ci, 2)
                    h0 = hh * HF
                    if ci == NCH - 1:
                        nc.vector.tensor_copy(
                            out=rawbuf[:, n, h0 : h0 + HF, :],
                            in_=pt[ci][:, :, 0:W],
                        )
                    else:
                        nc.scalar.activation(
                            out=rawbuf[:, n, h0 : h0 + HF, :],
                            in_=pt[ci][:, :, 0:W],
                            func=AF.Copy,
                        )
                    nc.vector.bn_stats(
                        out=stbuf[:, 6 * ci : 6 * (ci + 1)],
                        in_=rawbuf[:, n, h0 : h0 + HF, :].rearrange("p h w -> p (h w)"),
                    )

        def bn_scalars(ph, stbuf, g_col, b_col, fold_scale, bias_prescale=None):

            def vt(tag):
                return pp.tile([P, 1], F32, tag=f"{tag}{ph}", name=f"{tag}{ph}")

            cpk = pp.tile([P, 2], F32, tag=f"cpk{ph}", name=f"cpk{ph}")
            m2l = vt("m2l")
            nc.vector.bn_aggr(out=cpk[:], in_=stbuf[:])
            nc.vector.tensor_mul(out=m2l[:], in0=cpk[:, 0:1], in1=cpk[:, 0:1])
            nc.vector.tensor_add(out=cpk[:, 1:2], in0=cpk[:, 1:2], in1=m2l[:])
            t2p = tps.tile([2, P], F32, tag="t2p", name=f"t2p{ph}")
            nc.tensor.transpose(t2p[:], cpk[:], eyes[:])
            c2s = pp.tile([2, P], F32, tag=f"c2s{ph}", name=f"c2s{ph}")
            nc.scalar.activation(out=c2s[:], in_=t2p[:], func=AF.Copy)
            cin = dp.tile([2, P], F32, tag=f"cin{ph}", name=f"cin{ph}")
            cout = dp.tile([2 * N_CORES, P], F32, tag=f"cout{ph}", name=f"cout{ph}")
            nc.sync.dma_start(out=cin[:], in_=c2s[:])
            if ph == 1:
                nc.gpsimd.dma_start(out=wu_d, in_=wu_out[0, :])
            nc.gpsimd.collective_compute(
                "AllGather", OP.bypass, replica_groups=rg,
                ins=[cin.opt()], outs=[cout.opt()],
            )
            c16 = pp.tile([2 * N_CORES, P], F32, tag=f"c16_{ph}", name=f"c16_{ph}")
            nc.sync.dma_start(out=c16[:], in_=cout[:])
            stp = tps.tile([P, 2], F32, tag="stp", name=f"stp{ph}")
            nc.tensor.matmul(
                stp[:], c16[:], sms[:, 2 * (ph - 1) : 2 * ph],
                start=True, stop=True,
            )

            m2, u, s, r = vt("m2"), vt("u"), vt("s"), vt("r")
            mn = vt("mn")
            nc.vector.tensor_copy(out=mn[:], in_=stp[:, 0:1])
            nc.vector.tensor_mul(out=m2[:], in0=mn[:], in1=mn[:])
            nc.vector.tensor_sub(out=u[:], in0=stp[:, 1:2], in1=m2[:])
            nc.scalar.activation(out=s[:], in_=u[:], func=AF.Sqrt)
            nc.vector.reciprocal(out=r[:], in_=s[:])
            rgm, scaleA, b0, biasB = vt("rg"), vt("sA"), vt("b0"), vt("bB")
            nc.vector.tensor_mul(out=rgm[:], in0=r[:], in1=gbs[:, g_col : g_col + 1])
            if fold_scale == 1.0:
                scaleA = rgm
            else:
                nc.vector.tensor_scalar(
                    out=scaleA[:], in0=rgm[:], scalar1=fold_scale, scalar2=None, op0=OP.mult
                )
            if bias_prescale is None:
                nc.vector.tensor_mul(out=b0[:], in0=mn[:], in1=rgm[:])
            else:
                nc.vector.scalar_tensor_tensor(
                    out=b0[:], in0=mn[:], scalar=bias_prescale, in1=rgm[:],
                    op0=OP.mult, op1=OP.mult,
                )
            nc.vector.tensor_sub(
                out=biasB[:], in0=gbs[:, b_col : b_col + 1], in1=b0[:]
            )
            return scaleA, biasB

        st1 = pp.tile([P, NCH * 6], F32, tag="st1")
        conv([xh], w1s, raw1, st1)
        sA1, bB1 = bn_scalars(1, st1, 0, 1, 1.0, bias_prescale=15.0)

        for n in range(NIMG):
            bands = ((0, 16), (16, H)) if n <= 1 else ((0, H),)
            for r0, r1 in bands:
                if n % 2 == 0:
                    nc.vector._custom_dve(
                        QUANT_AFF_OP,
                        out=a1f[:, n, 1 + r0 : 1 + r1, 1 : 1 + W],
                        in0=raw1[:, n, r0:r1, :],
                        in1=bB1[:],
                        s0=sA1[:],
                        s1=15.0,
                        imm2=MAGIC,
                    )
                else:
                    u = fp.tile([P, r1 - r0, W], F32, tag="uq", name=f"uq{n}_{r0}")
                    nc.scalar.activation(
                        out=u[:], in_=raw1[:, n, r0:r1, :], func=AF.Relu,
                        bias=bB1[:], scale=sA1[:],
                    )
                    nc.vector._custom_dve(
                        QUANT_SIMPLE_OP,
                        out=a1f[:, n, 1 + r0 : 1 + r1, 1 : 1 + W],
                        in0=u[:],
                        s0=15.0,
                        s1=MAGIC,
                        imm2=0.0,
                    )

        st2 = pp.tile([P, NCH * 6], F32, tag="st2")
        conv2_dr(a1f, w2ps, raw2, st2)
        sA2, bB2 = bn_scalars(2, st2, 2, 3, 1.0 / 225.0)

        half = (H // 2) * W
        for n in range(NIMG):
            p1t = fp.tile([P, H * W], F32, tag="p1", name=f"p1_{n}")
            nc.scalar.activation(
                out=p1t[:],
                in_=raw2[:, n].rearrange("p h w -> p (h w)"),
                func=AF.Identity,
                bias=bB2[:],
                scale=sA2[:],
            )
            p1 = p1t[:]
            if n == NIMG - 1:
                for hh in range(2):
                    oh = fp.tile([P, half], F16, tag="og", name=f"og_{n}_{hh}")
                    nc.vector._custom_dve(
                        QUANT_RES_OP,
                        out=oh[:],
                        in0=xpad[:, n, 1 + hh * (H // 2) : 1 + (hh + 1) * (H // 2), 1 : 1 + W],
                        in1=p1[:, hh * half : (hh + 1) * half],
                        s0=15.0,
                        s1=MAGIC,
                        imm2=1.0 / 15.0,
                    )
                    q = (nc.sync, nc.scalar)[hh]
                    q.dma_start(
                        out=out_d[n, :, hh * (H // 2) : (hh + 1) * (H // 2), :],
                        in_=oh[:],
                    )
            else:
                og = fp.tile([P, H * W], F16, tag="og", name=f"og_{n}")
                nc.vector._custom_dve(
                    QUANT_RES_OP,
                    out=og[:],
                    in0=xpad[:, n, 1 : 1 + H, 1 : 1 + W],
                    in1=p1[:],
                    s0=15.0,
                    s1=MAGIC,
                    imm2=1.0 / 15.0,
                )
                q = (nc.sync, nc.gpsimd, nc.scalar)[n % 3]
                q.dma_start(out=out_d[n], in_=og[:])


_PROGRAM = None


def get_program():
    global _PROGRAM
    if _PROGRAM is None:
        nc = bacc.Bacc(
            "TRN2",
            target_bir_lowering=False,
            debug=False,
            enable_asserts=True,
            num_devices=N_CORES,
        )
        with tile.TileContext(nc, num_cores=N_CORES) as tc:
            _emit(nc, tc)
        nc.compile()
        _PROGRAM = nc
    return _PROGRAM


def make_in_maps(inputs):
    x = np.asarray(inputs["x"], np.float32)
    m1 = _quant_int(np.asarray(inputs["w1"], np.float32))
    mask = (np.asarray(inputs["mask2"], np.float32) > 0.5).astype(np.float32)
    m2 = _quant_int(np.asarray(inputs["w2"], np.float32) * mask)
    wq1 = _weights_lhsT(m1, np.float16)
    lhsT2 = _weights_lhsT(m2, np.float32)
    zero = np.zeros((P, P), np.float32)
    wq2p = np.stack(
        [
            np.stack(
                [lhsT2[:, ta, :], zero if tb is None else lhsT2[:, tb, :]], axis=1
            )
            for ta, tb in DR_PAIRS
        ],
        axis=1,
    ).astype(ml_dtypes.float8_e4m3)
    gb = np.stack(
        [
            np.asarray(inputs["gamma1"], np.float32),
            np.asarray(inputs["beta1"], np.float32) * np.float32(15.0),
            np.asarray(inputs["gamma2"], np.float32),
            np.asarray(inputs["beta2"], np.float32),
        ],
        axis=1,
    )
    gb = np.ascontiguousarray(gb)
    eye = np.eye(P, dtype=np.float32)
    smat = np.zeros((2 * N_CORES, 4), np.float32)
    for c in range(N_CORES):
        smat[2 * c + 0, 0] = 1.0 / (N_CORES * 15.0)
        smat[2 * c + 1, 1] = 1.0 / (N_CORES * 225.0)
        smat[2 * c + 0, 2] = 1.0 / (N_CORES * 225.0)
        smat[2 * c + 1, 3] = 1.0 / (N_CORES * 225.0 * 225.0)
    return [
        {
            "x": np.ascontiguousarray(x[NIMG * i : NIMG * (i + 1)]),
            "wq1": wq1,
            "wq2p": np.ascontiguousarray(wq2p),
            "gb": gb,
            "eye": eye,
            "smat": smat,
        }
        for i in range(N_CORES)
    ]


def run(inputs, **kwargs) -> bass_utils.BassKernelResults:
    nc = get_program()
    return bass_utils.run_bass_kernel_spmd(
        nc, make_in_maps(inputs), core_ids=list(range(N_CORES)), **kwargs
    )


def kernel(**inputs) -> np.ndarray:
    res = run(inputs)
    return np.concatenate(
        [res.results[i]["out"] for i in range(N_CORES)], axis=0
    ).astype(np.float32)


# revision 36
# speedup vs baseline: 1.0897x; 1.0897x over previous
import os
import sys

import numpy as np

for _p in ("/opt/trn_rl_repo",):
    if _p not in sys.path and os.path.isdir(_p):
        sys.path.insert(0, _p)

import ml_dtypes
import bass_rust

from concourse import bacc, mybir, tile
from concourse import bass_utils
from concourse import dve_ops
from concourse.dve_spec import C0, C1, C2, C3, Spec, Src0, Src1, lower, minn, relu
from concourse.dve_spec import _has_src1 as has_src1
from concourse.dve_spec import _spill_c3_to_src1
from concourse.dve_uop import DveOpSpec


def _register_dve_op(name, spec):
    for op in dve_ops.OPS:
        if op.name == name:
            return op
    row = dve_ops._CUSTOM_DVE_ROW_BASE + len(dve_ops.OPS)
    assert row < 0x20
    shas = {}
    for ver in ("v3", "v4"):
        shas[ver] = DveOpSpec(
            name=name, opcode=row, uops=lower(spec, ver=ver), rd1_en=has_src1(spec)
        ).sha(ver)
    op = dve_ops.DveOp(name, spec, subdim=False, uops_sha=shas)
    dve_ops.OPS.append(op)
    dve_ops.CUSTOM_DVE_SPECS[name] = spec
    dve_ops._SUB_OPCODE_FOR_NAME[name] = row
    return op


def _q(t, s0, s1, imm2):
    f = np.float32
    t = np.minimum(np.maximum(t, f(0.0)), f(s0)).astype(np.float32)
    t = (t + f(s1)).astype(np.float32)
    t = (t - f(s1)).astype(np.float32)
    return (t * f(imm2)).astype(np.float32)


QUANT_OP = _register_dve_op(
    "QUANT_CRS_ANT",
    Spec(
        body=(minn(relu(Src0 * C0), C0) + C1 - C1) * C2,
        reference=lambda in0, in1, s0, s1, imm2: _q(
            (in0.astype(np.float32) * np.float32(s0)).astype(np.float32), s0, s1, imm2
        ),
    ),
)

QUANT_AFF_OP = _register_dve_op(
    "QUANT_AFF_ANT",
    Spec(
        body=_spill_c3_to_src1(minn(relu(Src0 * C0 + C3), C1) + C2 - C2),
        reference=lambda in0, in1, s0, s1, imm2: (
            lambda t: (
                (np.minimum(np.maximum(t, np.float32(0.0)), np.float32(s1)).astype(np.float32)
                 + np.float32(imm2)).astype(np.float32)
                - np.float32(imm2)
            ).astype(np.float32)
        )(
            (
                in0.astype(np.float32).reshape(in0.shape[0], -1)
                * s0.astype(np.float32).reshape(-1, 1)
                + in1.astype(np.float32).reshape(-1, 1)
            ).astype(np.float32)
        ).reshape(in0.shape),
    ),
)

QUANT_SIMPLE_OP = _register_dve_op(
    "QUANT_SIM_ANT",
    Spec(
        body=minn(relu(Src0), C0) + C1 - C1,
        reference=lambda in0, in1, s0, s1, imm2: (
            lambda t: (
                (np.minimum(np.maximum(t, np.float32(0.0)), np.float32(s0)).astype(np.float32)
                 + np.float32(s1)).astype(np.float32)
                - np.float32(s1)
            ).astype(np.float32)
        )(in0.astype(np.float32)),
    ),
)

QUANT_RES_OP = _register_dve_op(
    "QUANT_RES_ANT",
    Spec(
        body=(minn(relu((Src0 + Src1) * C0), C0) + C1 - C1) * C2,
        reference=lambda in0, in1, s0, s1, imm2: _q(
            (
                (
                    in0.astype(np.float32).reshape(in0.shape[0], -1)
                    + in1.astype(np.float32).reshape(in1.shape[0], -1)
                ).astype(np.float32)
                * np.float32(s0)
            ).astype(np.float32),
            s0, s1, imm2,
        ).reshape(in0.shape),
    ),
)

N_CORES = 8
P = 128
NIMG = 8
H = W = 28
HP = 30
HR = 31
HF = 14
NCH = NIMG * 2
NRUN = HF * HP
HR2 = HR + 1
MAGIC = float(2 ** 23)
F32 = mybir.dt.float32
F16 = mybir.dt.float16
BF16 = mybir.dt.bfloat16
F8 = mybir.dt.float8e4
AF = mybir.ActivationFunctionType
OP = mybir.AluOpType
DR = mybir.MatmulPerfMode.DoubleRow

CONV_GROUP = 3

DR_PAIRS = [(0, 3), (1, 4), (2, 5), (6, None), (7, None), (8, None)]


def _quant_int(w: np.ndarray) -> np.ndarray:
    t = np.tanh(w.astype(np.float32))
    mx = np.max(np.abs(t))
    tq = t / (np.float32(2.0) * mx) + np.float32(0.5)
    j = np.round(tq * np.float32(15.0))
    return (np.float32(2.0) * j - np.float32(15.0)).astype(np.float32)


def _weights_lhsT(m: np.ndarray, dtype) -> np.ndarray:
    return np.ascontiguousarray(m.transpose(1, 2, 3, 0).reshape(P, 9, P)).astype(dtype)


def _emit(nc, tc):
    x_d = nc.dram_tensor("x", [NIMG, P, H, W], F32, kind="ExternalInput").ap()
    w1_d = nc.dram_tensor("wq1", [P, 9, P], F16, kind="ExternalInput").ap()
    w2p_d = nc.dram_tensor("wq2p", [P, 6, 2, P], F8, kind="ExternalInput").ap()
    gb_d = nc.dram_tensor("gb", [P, 4], F32, kind="ExternalInput").ap()
    eye_d = nc.dram_tensor("eye", [P, P], F32, kind="ExternalInput").ap()
    sm_d = nc.dram_tensor("smat", [2 * N_CORES, 4], F32, kind="ExternalInput").ap()
    out_d = nc.dram_tensor("out", [NIMG, P, H, W], F16, kind="ExternalOutput").ap()
    wu_d = nc.dram_tensor("wu", [P], F32, kind="ExternalOutput").ap()

    rg = [list(range(N_CORES))]

    with (
        tc.tile_pool(name="persist", bufs=1) as pp,
        tc.tile_pool(name="fin", bufs=4) as fp,
        tc.tile_pool(name="cpsum", bufs=6, space="PSUM") as pcp,
        tc.tile_pool(name="tpsum", bufs=1, space="PSUM") as tps,
        tc.tile_pool(name="dram", bufs=1, space="DRAM") as dp,
    ):
        wu_sb = pp.tile([2, P], F32, tag="wusb")
        wu_in = dp.tile([2, P], F32, tag="wuin", name="wuin")
        wu_out = dp.tile([N_CORES * 2, P], F32, tag="wuout", name="wuout")
        nc.vector.memset(wu_sb[:], 1.0)
        nc.gpsimd.dma_start(out=wu_in.opt(), in_=wu_sb[:])
        nc.gpsimd.collective_compute(
            "AllGather", OP.bypass, replica_groups=rg,
            ins=[wu_in.opt()], outs=[wu_out.opt()],
        )

        xpad = pp.tile([P, NIMG, HP, HP], F32, tag="xpad")
        xh = pp.tile([P, NIMG, HR, HP], F16, tag="xh")
        a1f = pp.tile([P, NIMG, HR2, HP], F8, tag="a1f")
        raw1 = pp.tile([P, NIMG, H, W], F32, tag="raw1")
        raw2 = pp.tile([P, NIMG, H, W], F32, tag="raw2")
        w1s = pp.tile([P, 9, P], F16, tag="w1s")
        w2ps = pp.tile([P, 6, 2, P], F8, tag="w2ps")
        gbs = pp.tile([P, 4], F32, tag="gbs")
        eyes = pp.tile([P, P], F32, tag="eyes")
        sms = pp.tile([2 * N_CORES, 4], F32, tag="sms")

        nc.vector.memset(xpad[:, :, 0, :], 0.0)
        nc.vector.memset(xpad[:, :, HP - 1, :], 0.0)
        nc.vector.memset(xpad[:, :, 1 : HP - 1, 0], 0.0)
        nc.vector.memset(xpad[:, :, 1 : HP - 1, HP - 1], 0.0)
        nc.vector.memset(xh[:, :, HR - 1, :], 0.0)
        nc.gpsimd.memset(a1f[:, :, 0, :], 0.0)
        nc.gpsimd.memset(a1f[:, :, HP - 1 :, :], 0.0)
        nc.gpsimd.memset(a1f[:, :, 1 : HP - 1, 0], 0.0)
        nc.gpsimd.memset(a1f[:, :, 1 : HP - 1, HP - 1], 0.0)

        for n in range(NIMG):
            q = nc.sync if n % 2 == 0 else nc.scalar
            q.dma_start(out=xpad[:, n, 1 : 1 + H, 1 : 1 + W], in_=x_d[n])
            nc.vector.tensor_copy(out=xh[:, n, 0:HP, :], in_=xpad[:, n])
            if n == 1:
                nc.scalar.dma_start(out=w1s[:], in_=w1_d)
        nc.sync.dma_start(out=w2ps[:], in_=w2p_d)
        nc.sync.dma_start(out=gbs[:], in_=gb_d)
        nc.sync.dma_start(out=eyes[:], in_=eye_d)
        nc.sync.dma_start(out=sms[:], in_=sm_d)

        warm = pp.tile([P, 1], F32, tag="warm")
        nc.vector.memset(warm[:], 1.0)
        nc.scalar.activation(out=warm[:], in_=warm[:], func=AF.Sqrt)

        def conv(pieces, wsb, rawbuf, stbuf):
            flats = {}
            for pi, piece in enumerate(pieces):
                for n in range(NIMG):
                    flats[(pi, n)] = piece[:, n].rearrange("p h w -> p (h w)")
            nmm = 9 * len(pieces)
            groups = [[0], [1, 2]] + [
                list(range(gs, min(gs + CONV_GROUP, NCH)))
                for gs in range(3, NCH, CONV_GROUP)
            ]
            for grp in groups:
                pt = {
                    ci: pcp.tile([P, HF, HP], F32, tag="cps", name=f"cps{ci}")
                    for ci in grp
                }
                for t in range(9):
                    dy, dx = divmod(t, 3)
                    for pi in range(len(pieces)):
                        k = t * len(pieces) + pi
                        for ci in grp:
                            n, hh = divmod(ci, 2)
                            off = (hh * HF + dy) * HP + dx
                            nc.tensor.matmul(
                                pt[ci][:],
                                wsb[:, t, :],
                                flats[(pi, n)][:, off : off + NRUN],
                                start=(k == 0),
                                stop=(k == nmm - 1),
                            )
                for ci in grp:
                    n, hh = divmod(ci, 2)
                    h0 = hh * HF
                    if ci == NCH - 1:
                        nc.vector.tensor_copy(
                            out=rawbuf[:, n, h0 : h0 + HF, :],
                            in_=pt[ci][:, :, 0:W],
                        )
                    else:
                        nc.scalar.activation(
                            out=rawbuf[:, n, h0 : h0 + HF, :],
                            in_=pt[ci][:, :, 0:W],
                            func=AF.Copy,
                        )
                    nc.vector.bn_stats(
                        out=stbuf[:, 6 * ci : 6 * (ci + 1)],
                        in_=rawbuf[:, n, h0 : h0 + HF, :].rearrange("p h w -> p (h w)"),
                    )

        def conv2_dr(piece, wp, rawbuf, stbuf):
            flats = {n: piece[:, n].rearrange("p h w -> p (h w)") for n in range(NIMG)}
            groups = [[0], [1, 2]] + [
                list(range(gs, min(gs + CONV_GROUP, NCH)))
                for gs in range(3, NCH, CONV_GROUP)
            ]
            for grp in groups:
                pt = {
                    ci: pcp.tile([P, HF, HP], F32, tag="cps", name=f"cps{ci}")
                    for ci in grp
                }
                for pi, (ta, _tb) in enumerate(DR_PAIRS):
                    dy, dx = divmod(ta, 3)
                    for ci in grp:
                        n, hh = divmod(ci, 2)
                        off = (hh * HF + dy) * HP + dx
                        mv = flats[n][:, off : off + NRUN].copy()
                        raw = mv.ap.to_list()
                        mv.ap = bass_rust.VecI64Pair([raw[0], [HP, 2], [1, NRUN]])
                        nc.tensor.matmul(
                            pt[ci][:], wp[:, pi], mv,
                            start=(pi == 0), stop=(pi == len(DR_PAIRS) - 1),
                            perf_mode=DR,
                        )
                for ci in grp:
                    n, hh = divmod(ci, 2)
                    h0 = hh * HF
                    if ci == NCH - 1:
                        nc.vector.tensor_copy(
                            out=rawbuf[:, n, h0 : h0 + HF, :],
                            in_=pt[ci][:, :, 0:W],
                        )
                    else:
                        nc.scalar.activation(
                            out=rawbuf[:, n, h0 : h0 + HF, :],
                            in_=pt[ci][:, :, 0:W],
                            func=AF.Copy,
                        )
                    nc.vector.bn_stats(
                        out=stbuf[:, 6 * ci : 6 * (ci + 1)],
                        in_=rawbuf[:, n, h0 : h0 + HF, :].rearrange("p h w -> p (h w)"),
                    )

        def bn_scalars(ph, stbuf, g_col, b_col, fold_scale, bias_prescale=None):

            def vt(tag):
                return pp.tile([P, 1], F32, tag=f"{tag}{ph}", name=f"{tag}{ph}")

            cpk = pp.tile([P, 2], F32, tag=f"cpk{ph}", name=f"cpk{ph}")
            m2l = vt("m2l")
            nc.vector.bn_aggr(out=cpk[:], in_=stbuf[:])
            nc.vector.tensor_mul(out=m2l[:], in0=cpk[:, 0:1], in1=cpk[:, 0:1])
            nc.vector.tensor_add(out=cpk[:, 1:2], in0=cpk[:, 1:2], in1=m2l[:])
            t2p = tps.tile([2, P], F32, tag="t2p", name=f"t2p{ph}")
            nc.tensor.transpose(t2p[:], cpk[:], eyes[:])
            c2s = pp.tile([2, P], F32, tag=f"c2s{ph}", name=f"c2s{ph}")
            nc.scalar.activation(out=c2s[:], in_=t2p[:], func=AF.Copy)
            cin = dp.tile([2, P], F32, tag=f"cin{ph}", name=f"cin{ph}")
            cout = dp.tile([2 * N_CORES, P], F32, tag=f"cout{ph}", name=f"cout{ph}")
            nc.sync.dma_start(out=cin[:], in_=c2s[:])
            if ph == 1:
                nc.gpsimd.dma_start(out=wu_d, in_=wu_out[0, :])
            nc.gpsimd.collective_compute(
                "AllGather", OP.bypass, replica_groups=rg,
                ins=[cin.opt()], outs=[cout.opt()],
            )
            c16 = pp.tile([2 * N_CORES, P], F32, tag=f"c16_{ph}", name=f"c16_{ph}")
            nc.sync.dma_start(out=c16[:], in_=cout[:])
            stp = tps.tile([P, 2], F32, tag="stp", name=f"stp{ph}")
            nc.tensor.matmul(
                stp[:], c16[:], sms[:, 2 * (ph - 1) : 2 * ph],
                start=True, stop=True,
            )

            m2, u, s, r = vt("m2"), vt("u"), vt("s"), vt("r")
            mn = vt("mn")
            nc.vector.tensor_copy(out=mn[:], in_=stp[:, 0:1])
            nc.vector.tensor_mul(out=m2[:], in0=mn[:], in1=mn[:])
            nc.vector.tensor_sub(out=u[:], in0=stp[:, 1:2], in1=m2[:])
            nc.scalar.activation(out=s[:], in_=u[:], func=AF.Sqrt)
            nc.vector.reciprocal(out=r[:], in_=s[:])
            rgm, scaleA, b0, biasB = vt("rg"), vt("sA"), vt("b0"), vt("bB")
            nc.vector.tensor_mul(out=rgm[:], in0=r[:], in1=gbs[:, g_col : g_col + 1])
            if fold_scale == 1.0:
                scaleA = rgm
            else:
                nc.vector.tensor_scalar(
                    out=scaleA[:], in0=rgm[:], scalar1=fold_scale, scalar2=None, op0=OP.mult
                )
            if bias_prescale is None:
                nc.vector.tensor_mul(out=b0[:], in0=mn[:], in1=rgm[:])
            else:
                nc.vector.scalar_tensor_tensor(
                    out=b0[:], in0=mn[:], scalar=bias_prescale, in1=rgm[:],
                    op0=OP.mult, op1=OP.mult,
                )
            nc.vector.tensor_sub(
                out=biasB[:], in0=gbs[:, b_col : b_col + 1], in1=b0[:]
            )
            return scaleA, biasB

        st1 = pp.tile([P, NCH * 6], F32, tag="st1")
        conv([xh], w1s, raw1, st1)
        sA1, bB1 = bn_scalars(1, st1, 0, 1, 1.0, bias_prescale=15.0)

        for n in range(NIMG):
            bands = ((0, 16), (16, H)) if n <= 1 else ((0, H),)
            for r0, r1 in bands:
                if n % 2 == 0:
                    nc.vector._custom_dve(
                        QUANT_AFF_OP,
                        out=a1f[:, n, 1 + r0 : 1 + r1, 1 : 1 + W],
                        in0=raw1[:, n, r0:r1, :],
                        in1=bB1[:],
                        s0=sA1[:],
                        s1=15.0,
                        imm2=MAGIC,
                    )
                else:
                    u = fp.tile([P, r1 - r0, W], F32, tag="uq", name=f"uq{n}_{r0}")
                    nc.scalar.activation(
                        out=u[:], in_=raw1[:, n, r0:r1, :], func=AF.Relu,
                        bias=bB1[:], scale=sA1[:],
                    )
                    nc.vector._custom_dve(
                        QUANT_SIMPLE_OP,
                        out=a1f[:, n, 1 + r0 : 1 + r1, 1 : 1 + W],
                        in0=u[:],
                        s0=15.0,
                        s1=MAGIC,
                        imm2=0.0,
                    )

        st2 = pp.tile([P, NCH * 6], F32, tag="st2")
        conv2_dr(a1f, w2ps, raw2, st2)
        sA2, bB2 = bn_scalars(2, st2, 2, 3, 1.0 / 225.0)

        half = (H // 2) * W
        for n in range(NIMG):
            p1t = fp.tile([P, H * W], F32, tag="p1", name=f"p1_{n}")
            nc.scalar.activation(
                out=p1t[:],
                in_=raw2[:, n].rearrange("p h w -> p (h w)"),
                func=AF.Identity,
                bias=bB2[:],
                scale=sA2[:],
            )
            p1 = p1t[:]
            if n == NIMG - 1:
                for hh in range(2):
                    oh = fp.tile([P, half], F16, tag="og", name=f"og_{n}_{hh}")
                    nc.vector._custom_dve(
                        QUANT_RES_OP,
                        out=oh[:],
                        in0=xpad[:, n, 1 + hh * (H // 2) : 1 + (hh + 1) * (H // 2), 1 : 1 + W],
                        in1=p1[:, hh * half : (hh + 1) * half],
                        s0=15.0,
                        s1=MAGIC,
                        imm2=1.0 / 15.0,
                    )
                    q = (nc.sync, nc.scalar)[hh]
                    q.dma_start(
                        out=out_d[n, :, hh * (H // 2) : (hh + 1) * (H // 2), :],
                        in_=oh[:],
                    )
            else:
                og = fp.tile([P, H * W], F16, tag="og", name=f"og_{n}")
                nc.vector._custom_dve(
                    QUANT_RES_OP,
                    out=og[:],
                    in0=xpad[:, n, 1 : 1 + H, 1 : 1 + W],
                    in1=p1[:],
                    s0=15.0,
                    s1=MAGIC,
                    imm2=1.0 / 15.0,
                )
                q = (nc.sync, nc.gpsimd, nc.scalar)[n % 3]
                q.dma_start(out=out_d[n], in_=og[:])


_PROGRAM = None


def get_program():
    global _PROGRAM
    if _PROGRAM is None:
        nc = bacc.Bacc(
            "TRN2",
            target_bir_lowering=False,
            debug=False,
            enable_asserts=True,
            num_devices=N_CORES,
        )
        with tile.TileContext(nc, num_cores=N_CORES) as tc:
            _emit(nc, tc)
        nc.compile()
        _PROGRAM = nc
    return _PROGRAM


def make_in_maps(inputs):
    x = np.asarray(inputs["x"], np.float32)
    m1 = _quant_int(np.asarray(inputs["w1"], np.float32))
    mask = (np.asarray(inputs["mask2"], np.float32) > 0.5).astype(np.float32)
    m2 = _quant_int(np.asarray(inputs["w2"], np.float32) * mask)
    wq1 = _weights_lhsT(m1, np.float16)
    lhsT2 = _weights_lhsT(m2, np.float32)
    zero = np.zeros((P, P), np.float32)
    wq2p = np.stack(
        [
            np.stack(
                [lhsT2[:, ta, :], zero if tb is None else lhsT2[:, tb, :]], axis=1
            )
            for ta, tb in DR_PAIRS
        ],
        axis=1,
    ).astype(ml_dtypes.float8_e4m3)
    gb = np.stack(
        [
            np.asarray(inputs["gamma1"], np.float32),
            np.asarray(inputs["beta1"], np.float32) * np.float32(15.0),
            np.asarray(inputs["gamma2"], np.float32),
            np.asarray(inputs["beta2"], np.float32),
        ],
        axis=1,
    )
    gb = np.ascontiguousarray(gb)
    eye = np.eye(P, dtype=np.float32)
    smat = np.zeros((2 * N_CORES, 4), np.float32)
    for c in range(N_CORES):
        smat[2 * c + 0, 0] = 1.0 / (N_CORES * 15.0)
        smat[2 * c + 1, 1] = 1.0 / (N_CORES * 225.0)
        smat[2 * c + 0, 2] = 1.0 / (N_CORES * 225.0)
        smat[2 * c + 1, 3] = 1.0 / (N_CORES * 225.0 * 225.0)
    return [
        {
            "x": np.ascontiguousarray(x[NIMG * i : NIMG * (i + 1)]),
            "wq1": wq1,
            "wq2p": np.ascontiguousarray(wq2p),
            "gb": gb,
            "eye": eye,
            "smat": smat,
        }
        for i in range(N_CORES)
    ]


def run(inputs, **kwargs) -> bass_utils.BassKernelResults:
    nc = get_program()
    return bass_utils.run_bass_kernel_spmd(
        nc, make_in_maps(inputs), core_ids=list(range(N_CORES)), **kwargs
    )


def kernel(**inputs) -> np.ndarray:
    res = run(inputs)
    return np.concatenate(
        [res.results[i]["out"] for i in range(N_CORES)], axis=0
    ).astype(np.float32)
